# revision 23
# baseline (speedup 1.0000x reference)
"""Trainium2 Bass kernel for the ChiSq (histogram_binning) problem.

Pipeline per core (128 of the 1024 (batch,channel) rows, pure data parallel):
  1. rfft(16384) of template & strain via 2-stage Cooley-Tukey matmul FFT
     (128x128 DFT stages on the PE array + elementwise twiddle).
  2. ph = c*|Ht|^2, cross = c*Re(conj(Ht) Hs) on the [k2, (row,k1)] grid.
  3. Unflatten to [row, k] layout, cumsum (tensor_tensor_scan), threshold
     masked sums -> per-bin SNR -> chi-square.
"""
import numpy as np
from contextlib import ExitStack

import concourse.bass as bass
import concourse.tile as tile
from concourse import bacc, mybir
from concourse.bass_utils import run_bass_kernel_spmd

F32 = mybir.dt.float32
F32R = mybir.dt.float32r


def _r(ap):
    """View an fp32 AP as float32r for fast PE matmuls (same bits)."""
    return ap.bitcast(F32R)


def _round_f32r(a: np.ndarray) -> np.ndarray:
    """Round fp32 values to fp32r (11-bit mantissa, RNE) on the host."""
    u = np.ascontiguousarray(a, np.float32).view(np.uint32)
    lsb = (u >> np.uint32(12)) & np.uint32(1)
    u2 = (u + np.uint32(0x7FF) + lsb) & np.uint32(0xFFFFF000)
    return u2.view(np.float32)

# problem constants (hardcoded; kernel.py must be self-contained)
SAMPLE_RATE = 2048.0
FFTLENGTH = 8.0
NUM_BINS = 16
N = int(FFTLENGTH * SAMPLE_RATE)      # 16384
NF = N // 2 + 1                        # 8193
DF = 1.0 / FFTLENGTH
CSCALE = 4.0 * DF / (SAMPLE_RATE ** 2)

NCORES = 8
ROWS = 128            # rows per core (1024 total)
R = 4                 # rows per chunk
NCHUNK = ROWS // R    # 32
NK2 = 65              # k2 in [0, 64]
NPAD = 128 * NK2      # 8320


def _make_consts():
    n1 = np.arange(128)
    ang1 = 2 * np.pi * np.outer(n1, n1) / 128.0
    c128 = np.cos(ang1).astype(np.float32)         # symmetric
    s128 = np.sin(ang1).astype(np.float32)
    k1 = np.arange(128)
    n2 = np.arange(128)
    angw = 2 * np.pi * np.outer(k1, n2) / float(N)
    wr = np.cos(angw).astype(np.float32)           # TW = wr + i*wi
    wi = (-np.sin(angw)).astype(np.float32)
    wr_rep = np.tile(wr, (1, R)).astype(np.float32)
    wi_rep = np.tile(wi, (1, R)).astype(np.float32)
    k2 = np.arange(NK2)
    ange = 2 * np.pi * np.outer(n2, k2) / 128.0
    sc = np.float32(np.sqrt(CSCALE))
    er = (np.cos(ange) * sc).astype(np.float32)    # E = er + i*ei, pre-scaled
    ei = (-np.sin(ange) * sc).astype(np.float32)
    eineg = (-ei).astype(np.float32)
    ident = np.eye(128, dtype=np.float32)
    mfrac = np.tile((np.arange(1, 16, dtype=np.float32) / 16.0)[None, :], (128, 1))
    d = dict(c128=c128, s128=s128, wr_rep=wr_rep, wi_rep=wi_rep,
             er=er, ei=ei, eineg=eineg, ident=ident, mfrac=mfrac)
    for k in ("c128", "s128", "er", "ei", "eineg", "ident"):
        d[k] = _round_f32r(d[k])
    return d


def _build_program():
    nc = bacc.Bacc("TRN2", target_bir_lowering=False, debug=False,
                   enable_asserts=False, num_devices=NCORES)
    t_in = nc.dram_tensor("t_in", [ROWS, N], F32, kind="ExternalInput").ap()
    s_in = nc.dram_tensor("s_in", [ROWS, N], F32, kind="ExternalInput").ap()
    consts = _make_consts()
    MM_CONSTS = {"c128", "s128", "er", "ei", "eineg", "ident"}
    capz = {k: nc.dram_tensor(k, list(v.shape),
                              F32R if k in MM_CONSTS else F32,
                              kind="ExternalInput").ap()
            for k, v in consts.items()}
    out = nc.dram_tensor("chisq_out", [ROWS, 1], F32, kind="ExternalOutput").ap()

    AL = mybir.AluOpType

    with tile.TileContext(nc, trace_sim=False) as tc, ExitStack() as ctx:
        cpool = ctx.enter_context(tc.tile_pool(name="consts", bufs=1))
        big = ctx.enter_context(tc.tile_pool(name="big", bufs=1))
        inp = ctx.enter_context(tc.tile_pool(name="inp", bufs=2))
        work = ctx.enter_context(tc.tile_pool(name="work", bufs=1))
        ps1 = ctx.enter_context(tc.tile_pool(name="ps1", bufs=1, space="PSUM"))
        ps2 = ctx.enter_context(tc.tile_pool(name="ps2", bufs=1, space="PSUM"))
        ps3 = ctx.enter_context(tc.tile_pool(name="ps3", bufs=1, space="PSUM"))

        ct = {}
        for k, v in consts.items():
            ct[k] = cpool.tile(list(v.shape),
                               F32R if k in MM_CONSTS else F32, tag=k, name=k)
            nc.sync.dma_start(ct[k][:], capz[k][:])

        PH = big.tile([128, NPAD], F32, tag="PH")
        CR = big.tile([128, NPAD], F32, tag="CR")

        def fft_signal(x_dram, r0, out_psum, sig):
            """FFT chunk rows [r0, r0+R) of one signal.
            Returns (Xr_ps, Xi_ps) PSUM tiles [65, R*128] on grid [k2,(row,k1)]."""
            xt = inp.tile([128, R * 128], F32, tag="xt_" + sig, name="xt_" + sig)
            nc.sync.dma_start(
                _r(xt[:].rearrange("p (r f) -> p r f", r=R)),
                _r(x_dram[r0:r0 + R, :].rearrange("r (p f) -> p r f", p=128)))
            yc = ps1.tile([128, R * 128], F32, tag="yc_" + sig, name="yc_" + sig)
            ys = ps1.tile([128, R * 128], F32, tag="ys_" + sig, name="ys_" + sig)
            nc.tensor.matmul(yc[:], _r(ct["c128"][:]), _r(xt[:]), start=True, stop=True)
            nc.tensor.matmul(ys[:], _r(ct["s128"][:]), _r(xt[:]), start=True, stop=True)
            # twiddle: Zr = yc*wr + ys*wi ; Zi = yc*wi - ys*wr
            t1 = work.tile([128, R * 128], F32, tag="t1" + sig, name="t1" + sig)
            t2 = work.tile([128, R * 128], F32, tag="t2" + sig, name="t2" + sig)
            t3 = work.tile([128, R * 128], F32, tag="t3" + sig, name="t3" + sig)
            t4 = work.tile([128, R * 128], F32, tag="t4" + sig, name="t4" + sig)
            zr = work.tile([128, R * 128], F32, tag="zr" + sig, name="zr" + sig)
            zi = work.tile([128, R * 128], F32, tag="zi" + sig, name="zi" + sig)
            nc.vector.tensor_tensor(t1[:], yc[:], ct["wr_rep"][:], op=AL.mult)
            nc.vector.tensor_tensor(t2[:], ys[:], ct["wi_rep"][:], op=AL.mult)
            nc.gpsimd.tensor_tensor(_r(zr[:]), t1[:], t2[:], op=AL.add)
            nc.vector.tensor_tensor(t3[:], yc[:], ct["wi_rep"][:], op=AL.mult)
            nc.vector.tensor_tensor(t4[:], ys[:], ct["wr_rep"][:], op=AL.mult)
            nc.gpsimd.tensor_tensor(_r(zi[:]), t3[:], t4[:], op=AL.subtract)
            # transpose each row's [k1, n2] block -> [n2, k1]
            zrt = ps2.tile([128, R * 128], F32, tag="zrt", name="zrt" + sig)
            zit = ps2.tile([128, R * 128], F32, tag="zit", name="zit" + sig)
            for r in range(R):
                sl = slice(r * 128, (r + 1) * 128)
                nc.tensor.transpose(_r(zrt[:, sl]), _r(zr[:, sl]), _r(ct["ident"][:]))
                nc.tensor.transpose(_r(zit[:, sl]), _r(zi[:, sl]), _r(ct["ident"][:]))
            zrt_sb = work.tile([128, R * 128], F32, tag="zrt_sb" + sig, name="zrt_sb" + sig)
            zit_sb = work.tile([128, R * 128], F32, tag="zit_sb" + sig, name="zit_sb" + sig)
            nc.scalar.copy(_r(zrt_sb[:]), zrt[:])
            nc.scalar.copy(_r(zit_sb[:]), zit[:])
            # stage 3: X = Z @ E  (complex), out [k2, (row,k1)]
            xr = out_psum.tile([NK2, R * 128], F32, tag="xr", name="xr" + sig)
            xi = out_psum.tile([NK2, R * 128], F32, tag="xi", name="xi" + sig)
            nc.tensor.matmul(xr[:], _r(ct["er"][:]), _r(zrt_sb[:]), start=True, stop=False)
            nc.tensor.matmul(xr[:], _r(ct["eineg"][:]), _r(zit_sb[:]), start=False, stop=True)
            nc.tensor.matmul(xi[:], _r(ct["ei"][:]), _r(zrt_sb[:]), start=True, stop=False)
            nc.tensor.matmul(xi[:], _r(ct["er"][:]), _r(zit_sb[:]), start=False, stop=True)
            return xr, xi

        for ci in range(NCHUNK):
            r0 = ci * R
            xrt, xit = fft_signal(t_in, r0, ps3, "t")
            xrt_sb = work.tile([NK2, R * 128], F32, tag="xrt_sb", name="xrt_sb")
            xit_sb = work.tile([NK2, R * 128], F32, tag="xit_sb", name="xit_sb")
            nc.scalar.copy(xrt_sb[:], xrt[:])
            nc.scalar.copy(xit_sb[:], xit[:])
            xrs, xis = fft_signal(s_in, r0, ps3, "s")
            # ph = xrt^2 + xit^2 ; cross = xrt*xrs + xit*xis   (c-scaled via E)
            u1 = work.tile([NK2, R * 128], F32, tag="u1", name="u1")
            u2 = work.tile([NK2, R * 128], F32, tag="u2", name="u2")
            u3 = work.tile([NK2, R * 128], F32, tag="u3", name="u3")
            u4 = work.tile([NK2, R * 128], F32, tag="u4", name="u4")
            phc = work.tile([NK2, R * 128], F32, tag="phc", name="phc")
            crc = work.tile([NK2, R * 128], F32, tag="crc", name="crc")
            nc.scalar.square(u1[:], xrt_sb[:])
            nc.scalar.square(u2[:], xit_sb[:])
            nc.gpsimd.tensor_tensor(phc[:], u1[:], u2[:], op=AL.add)
            nc.vector.tensor_tensor(u3[:], xrt_sb[:], xrs[:], op=AL.mult)
            nc.vector.tensor_tensor(u4[:], xit_sb[:], xis[:], op=AL.mult)
            nc.gpsimd.tensor_tensor(crc[:], u3[:], u4[:], op=AL.add)
            # unflatten to [row, k] big tiles
            for r in range(R):
                sl = slice(r * 128, (r + 1) * 128)
                row = r0 + r
                nc.sync.dma_start(
                    PH[row:row + 1, :].rearrange("o (k f) -> o k f", k=NK2),
                    phc[:, sl].rearrange("k f -> k () f"))
                nc.sync.dma_start(
                    CR[row:row + 1, :].rearrange("o (k f) -> o k f", k=NK2),
                    crc[:, sl].rearrange("k f -> k () f"))

        # ---- binning ----
        nc.vector.memset(PH[:, NF:NPAD], 0.0)
        nc.vector.memset(CR[:, NF:NPAD], 0.0)
        nc.vector.tensor_tensor_scan(PH[:], PH[:], PH[:], 0.0, AL.add, AL.bypass)
        CH = PH
        th = CH[:, NF - 1:NF]
        tvals = cpool.tile([128, 15], F32, tag="tvals")
        nc.vector.tensor_scalar(tvals[:], ct["mfrac"][:], th, None, op0=AL.mult)
        stot = cpool.tile([128, 1], F32, tag="stot")
        nc.vector.tensor_reduce(stot[:], CR[:, 0:NF], op=AL.add,
                                axis=mybir.AxisListType.X)
        G = cpool.tile([128, 17], F32, tag="G")
        nc.vector.memset(G[:], 0.0)
        nc.vector.tensor_tensor(G[:, 0:1], stot[:], CR[:, 0:1], op=AL.subtract)
        scratch = big.tile([128, NPAD - 1], F32, tag="scratch")
        for m in range(1, 16):
            nc.vector.scalar_tensor_tensor(
                scratch[:, 0:NPAD - 1], CH[:, 0:NPAD - 1], tvals[:, m - 1:m],
                CR[:, 1:NPAD], AL.is_gt, AL.mult,
                accum_out=G[:, m:m + 1])
        snrb = cpool.tile([128, 16], F32, tag="snrb")
        nc.vector.tensor_tensor(snrb[:], G[:, 0:16], G[:, 1:17], op=AL.subtract)
        s16 = cpool.tile([128, 1], F32, tag="s16")
        nc.vector.tensor_scalar_mul(s16[:], stot[:], 1.0 / 16.0)
        ee = cpool.tile([128, 16], F32, tag="ee")
        nc.vector.tensor_scalar(ee[:], snrb[:], s16[:], None, op0=AL.subtract)
        esq = cpool.tile([128, 16], F32, tag="esq")
        nc.vector.tensor_tensor(esq[:], ee[:], ee[:], op=AL.mult)
        ssum = cpool.tile([128, 1], F32, tag="ssum")
        nc.vector.tensor_reduce(ssum[:], esq[:], op=AL.add,
                                axis=mybir.AxisListType.X)
        rth = cpool.tile([128, 1], F32, tag="rth")
        nc.vector.reciprocal(rth[:], th)
        chq = cpool.tile([128, 1], F32, tag="chq")
        nc.vector.tensor_tensor(chq[:], ssum[:], rth[:], op=AL.mult)
        nc.vector.tensor_scalar_mul(chq[:], chq[:], float(NUM_BINS) / (NUM_BINS - 1))
        nc.sync.dma_start(out[:], chq[:])

    nc.compile()
    return nc, consts


_CACHE = {}


def kernel(template: np.ndarray, strain: np.ndarray) -> np.ndarray:
    if "nc" not in _CACHE:
        _CACHE["nc"], _CACHE["consts"] = _build_program()
    nc, consts = _CACHE["nc"], _CACHE["consts"]

    t = _round_f32r(np.asarray(template, np.float32).reshape(1024, N))
    s = _round_f32r(np.asarray(strain, np.float32).reshape(1024, N))
    in_maps = []
    for c in range(NCORES):
        m = {"t_in": t[c * ROWS:(c + 1) * ROWS], "s_in": s[c * ROWS:(c + 1) * ROWS]}
        m.update(consts)
        in_maps.append(m)
    res = run_bass_kernel_spmd(nc, in_maps, list(range(NCORES)))
    outs = [res.results[c]["chisq_out"].reshape(ROWS) for c in range(NCORES)]
    full = np.concatenate(outs).astype(np.float32)
    return full.reshape(512, 2)


if __name__ == "__main__":
    rng = np.random.default_rng(0)
    tpl = rng.standard_normal((512, 2, N), dtype=np.float32)
    st = rng.standard_normal((512, 2, N), dtype=np.float32)
    print(kernel(tpl, st)[:3])

